# revision 1
# baseline (speedup 1.0000x reference)
"""Multi-head dense attention (no softmax) on 8 Trainium2 NeuronCores.

Math (per batch b, head h with head_dim d=64):
    q   = x @ W^T                      # [S, H] projection
    out_h = (q_h x_h^T) x_h            # naive: O(S^2 d) with an SxS temp
          = q_h (x_h^T x_h)            # reassociated: Gram matrix G_h [d, d]
The reassociation is exact (same sum, different order) and collapses the
FLOPs ~5x while removing the SxS intermediate entirely.

Sharding: core c handles batch b = c//2 and head-group hg = c%2 (8 heads,
512 output columns). Cores are fully independent (no collectives).

Device layout per core (all inputs fp16; W is pre-scaled by 1024 on the
host so its sigma~9e-5 entries clear fp16's subnormal cutoff; the Gram
tile copy multiplies by 1/1024 to undo it):
    xT  [1024, 2048]  x[b] transposed (host-prepped)  - projection operands
    xn  [2048, 512]   x[b] natural, this head-group's columns - Gram operands
    wT  [1024, 512]   1024 * W rows of this head-group, transposed (k-major)
    outT [512, 2048]  output transposed (fp16); host transposes/upcasts back

Schedule (fp16 matmuls: 1 cycle/row at N=512, fp32 PSUM; the kernel is
co-limited by PE throughput and the ~300 GB/s per-core DMA fabric, so
the projection's k-contraction is split in half around the DMA stream):
    1. halfA: qA' = (1024 W) x^T over k-tiles 0-1, while those tiles land.
    2. Gram:  per head-pair p, psum += xn_p^T xn_p over 16 s-tiles; the two
       64x64 diagonal blocks are copied (scaled 1/1024) into a zeroed
       block-diagonal fp16 lhsT tile. Runs while xn + xT 2-7 stream in.
    3. halfB: k-tiles 2-7; the PSUM->SBUF drain is a tensor_add that folds
       qA' in (qT' = psum + qA'), and after each m-tile p the output stage
       emits outT_p = Gbd_p^T qT'_p as one N=512 matmul per s-chunk, then
       copies PSUM->SBUF (fp16) and stores on the sync/gpsimd rings.
wT is host-blocked m-tile-major so every weight descriptor reads 2KB
contiguous lines.
All input descriptors ride the Activation engine's DMA ring in exact
consumption order; stores alternate the Sync and GpSimd rings.
"""

import numpy as np

B, S, H = 4, 2048, 1024
N_HEADS = 16
HD = H // N_HEADS  # 64
N_CORES = 8
MG = H // 2        # 512 output columns per core
P = 128
KT = H // P        # 8 k-tiles
ST = S // P        # 16 s-tiles
MT = MG // P       # 4 m-tiles == head pairs
SC = S // 512      # 4 s-chunks
W_SCALE = 1024.0
KT_A = 2             # k-tiles in the first projection half

_NC_CACHE = {}


def _build_nc():
    import concourse.mybir as mybir
    from concourse import bacc
    from concourse.tile import TileContext

    f32 = mybir.dt.float32
    f16 = mybir.dt.float16

    nc = bacc.Bacc()
    xT_d = nc.declare_dram_parameter("xT", [H, S], f16, isOutput=False)
    xn_d = nc.declare_dram_parameter("xn", [S, MG], f16, isOutput=False)
    wT_d = nc.declare_dram_parameter("wT", [MT * P, KT * P], f16, isOutput=False)
    outT_d = nc.declare_dram_parameter("outT", [MG, S], f16, isOutput=True)

    xT_t = xT_d.rearrange("(kt p) s -> p kt s", p=P)   # [128, 8, 2048]
    xn_t = xn_d.rearrange("(st p) m -> p st m", p=P)   # [128, 16, 512]
    wT_t = wT_d.rearrange("(mt p) (kt m) -> p mt kt m", p=P, m=P)  # [128, 4, 8, 128]

    with TileContext(nc) as tc:
        with (
            tc.tile_pool(name="big", bufs=1) as big,
            tc.tile_pool(name="gp", bufs=1) as gpool,
            tc.tile_pool(name="stage", bufs=4) as stage,
            tc.tile_pool(name="ps_q", bufs=1, space="PSUM") as ps_q,
        ):
            xT_sb = big.tile([P, KT, S], f16, tag="xT")
            xn_sb = big.tile([P, ST, MG], f16, tag="xn")
            wT_sb = big.tile([P, MT, KT, P], f16, tag="wT")
            qA_sb = big.tile([P, MT, S], f16, tag="qA")
            qB_sb = big.tile([P, MT, S], f16, tag="qB")

            # Single Activation-engine ring, descriptors in consumption
            # order: wT m-tile 0, xT k-tiles 0-1 (first projection half),
            # remaining wT, xn (Gram runs in the middle), xT k-tiles 2-7
            # (second half). Output stores alternate Sync/GpSimd rings.
            nc.scalar.dma_start(out=wT_sb[:, 0], in_=wT_t[:, 0])
            nc.scalar.dma_start(out=xT_sb[:, 0], in_=xT_t[:, 0])
            nc.scalar.dma_start(out=xT_sb[:, 1], in_=xT_t[:, 1])
            nc.scalar.dma_start(out=wT_sb[:, 1:], in_=wT_t[:, 1:])
            nc.scalar.dma_start(out=xn_sb[:, :8], in_=xn_t[:, :8])
            nc.scalar.dma_start(out=xn_sb[:, 8:], in_=xn_t[:, 8:])
            for kt in range(KT_A, KT):
                nc.scalar.dma_start(out=xT_sb[:, kt], in_=xT_t[:, kt])

            def proj_half(kts, q_sb, phase):
                for mt in range(MT):
                    psqs = [
                        ps_q.tile(
                            [P, 512], f32, tag=f"psq{sc}", name=f"psq{phase}_{mt}_{sc}"
                        )
                        for sc in range(SC)
                    ]
                    for n, kt in enumerate(kts):
                        for sc in range(SC):
                            nc.tensor.matmul(
                                psqs[sc],
                                lhsT=wT_sb[:, mt, kt],
                                rhs=xT_sb[:, kt, sc * 512:(sc + 1) * 512],
                                start=(n == 0),
                                stop=(n == len(kts) - 1),
                            )
                    for sc in range(SC):
                        nc.vector.tensor_copy(
                            out=q_sb[:, mt, sc * 512:(sc + 1) * 512], in_=psqs[sc]
                        )

            # ---- First projection half: only needs wT + xT k-tiles 0-1.
            proj_half(range(KT_A), qA_sb, "A")

            # ---- Gram stage: needs xn, which lands mid-stream.
            gbd = []
            with tc.tile_pool(name="ps_g", bufs=2, space="PSUM") as ps_g:
                for p_i in range(MT):
                    psg = ps_g.tile([P, P], f32, tag="psg", name=f"psg{p_i}")
                    xp = xn_sb[:, :, p_i * P:(p_i + 1) * P]
                    for i in range(ST):
                        nc.tensor.matmul(
                            psg,
                            lhsT=xp[:, i],
                            rhs=xp[:, i],
                            start=(i == 0),
                            stop=(i == ST - 1),
                        )
                    g = gpool.tile([P, P], f16, tag=f"g{p_i}", name=f"g{p_i}")
                    nc.vector.memset(g, 0.0)
                    nc.vector.tensor_scalar_mul(
                        out=g[0:HD, 0:HD], in0=psg[0:HD, 0:HD], scalar1=1.0 / W_SCALE
                    )
                    nc.vector.tensor_scalar_mul(
                        out=g[HD:P, HD:P], in0=psg[HD:P, HD:P], scalar1=1.0 / W_SCALE
                    )
                    gbd.append(g)

            with tc.tile_pool(name="ps_o", bufs=4, space="PSUM") as ps_o:
                # ---- Second projection half (xT k-tiles 4-7), with pair p's
                # output stage emitted one m-tile behind: out = G qA + G qB
                # accumulated in PSUM (linearity), so the halves never need to
                # be added explicitly.
                def emit_out(p_i):
                    for sc in range(SC):
                        pso = ps_o.tile([P, 512], f32, tag="pso", name=f"pso{p_i}_{sc}")
                        nc.tensor.matmul(
                            pso,
                            lhsT=gbd[p_i],
                            rhs=qB_sb[:, p_i, sc * 512:(sc + 1) * 512],
                            start=True,
                            stop=True,
                        )
                        ot = stage.tile([P, 512], f16, tag="ot", name=f"ot{p_i}_{sc}")
                        nc.vector.tensor_copy(out=ot, in_=pso)
                        st_eng = nc.sync if sc % 2 == 0 else nc.gpsimd
                        st_eng.dma_start(
                            out=outT_d[p_i * P:(p_i + 1) * P, sc * 512:(sc + 1) * 512],
                            in_=ot,
                        )

                for mt in range(MT):
                    psqs = [
                        ps_q.tile([P, 512], f32, tag=f"psq{sc}", name=f"psqB{mt}_{sc}")
                        for sc in range(SC)
                    ]
                    kts_b = range(KT_A, KT)
                    for n, kt in enumerate(kts_b):
                        for sc in range(SC):
                            nc.tensor.matmul(
                                psqs[sc],
                                lhsT=wT_sb[:, mt, kt],
                                rhs=xT_sb[:, kt, sc * 512:(sc + 1) * 512],
                                start=(n == 0),
                                stop=(n == KT // 2 - 1),
                            )
                    for sc in range(SC):
                        nc.vector.tensor_add(
                            out=qB_sb[:, mt, sc * 512:(sc + 1) * 512],
                            in0=psqs[sc],
                            in1=qA_sb[:, mt, sc * 512:(sc + 1) * 512],
                        )
                    emit_out(mt)
    nc.compile()
    return nc


def _get_nc():
    if "nc" not in _NC_CACHE:
        _NC_CACHE["nc"] = _build_nc()
    return _NC_CACHE["nc"]


def make_in_maps(hidden_states, queries_weight):
    hs = np.ascontiguousarray(np.asarray(hidden_states, dtype=np.float32))
    w = np.ascontiguousarray(np.asarray(queries_weight, dtype=np.float32))
    in_maps = []
    for c in range(N_CORES):
        b, hg = divmod(c, 2)
        xb = hs[b]
        in_maps.append({
            "xT": np.ascontiguousarray(xb.T).astype(np.float16),
            "xn": np.ascontiguousarray(xb[:, hg * MG:(hg + 1) * MG]).astype(
                np.float16
            ),
            "wT": np.ascontiguousarray(
                (w[hg * MG:(hg + 1) * MG, :].T * W_SCALE)
                .reshape(KT, P, MT, P)
                .transpose(2, 1, 0, 3)
                .reshape(MT * P, KT * P)
            ).astype(np.float16),
        })
    return in_maps


def assemble_output(results):
    out = np.empty((B, S, H), dtype=np.float32)
    for c in range(N_CORES):
        b, hg = divmod(c, 2)
        out[b, :, hg * MG:(hg + 1) * MG] = results[c]["outT"].T.astype(np.float32)
    return out


def kernel(hidden_states, queries_weight):
    from concourse.bass_utils import run_bass_kernel_spmd

    in_maps = make_in_maps(hidden_states, queries_weight)
    res = run_bass_kernel_spmd(
        _get_nc(), in_maps, core_ids=list(range(N_CORES))
    ).results
    return assemble_output(res)


if __name__ == "__main__":
    x = np.random.randn(B, S, H).astype(np.float32)
    w = np.random.randn(H, H).astype(np.float32) * 1e-4
    out = kernel(x, w)
    print(out.shape, out.dtype)



# revision 5
# speedup vs baseline: 1.0433x; 1.0433x over previous
"""Multi-head dense attention (no softmax) on 8 Trainium2 NeuronCores.

Math (per batch b, head h with head_dim d=64):
    q   = x @ W^T                      # [S, H] projection
    out_h = (q_h x_h^T) x_h            # naive: O(S^2 d) with an SxS temp
          = q_h (x_h^T x_h)            # reassociated: Gram matrix G_h [d, d]
The reassociation is exact (same sum, different order) and collapses the
FLOPs ~5x while removing the SxS intermediate entirely.

Sharding: core c handles batch b = c//2 and head-group hg = c%2 (8 heads,
512 output columns). Cores are fully independent (no collectives).

v2 schedule (vs v1 baseline at ~62 us):
  - Single-pass projection: psum accumulates all 8 k-tiles per (sc, mt),
    ONE fp32->fp16 drain per tile (v1 drained each half => 2x vector work).
  - Drains alternate between the Vector (DVE) and Scalar (Activation)
    engines so neither sits on the critical path.
  - Gram runs in fp8e4 (e4m3) with DoubleRow perf mode: 2 s-tiles per
    instruction at 0.5 cyc/row => 4x fewer tensor cycles than fp16, and
    xn ships as 1 MiB instead of 2.
  - All input DMA rides ONE Sync-engine ring in exact consumption order,
    interleaved wT[kt] / xT[sc0,kt] so the first projection matmul starts
    ~1 us in and the sc0 k-loop streams behind the DMA. Later s-chunks
    (1 MiB each) arrive ahead of their compute. Stores ride GpSimd.
  - Out-stage (out_p[:,sc] = G_p^T q_p[:,sc]) is emitted per s-chunk right
    after its q drain, so the tail is one matmul + drain + store.
  - A short warmup matmul burst at t=0 spins the PE p-state up during the
    initial DMA latency.

Per-core tensor work: 128 proj matmuls (N=512) + 32 DR gram (N=128) +
16 out (N=512) ~= 75.8k PE rows ~= 31.6 us @ 2.4 GHz; DMA 7 MiB ~= 22 us
on one 323 GB/s ring; both overlap => target ~35 us.

Device layout per core:
    xT  [SC*KT*128, 512] f16  x[b]^T blocked (sc,kt)-major, 128KiB/chunk
    xn  [2048, 512]      f8e4 x[b] natural, head-group cols (Gram operand)
    wT  [KT*128, 512]    f16  1024*W_hg^T, k-major (kt blocks contiguous)
    outB [MT*SC*128, 512] f16 out^T blocked (mt,sc); host reassembles
"""

import numpy as np

B, S, H = 4, 2048, 1024
N_HEADS = 16
HD = H // N_HEADS  # 64
N_CORES = 8
MG = H // 2        # 512 output columns per core
P = 128
KT = H // P        # 8 k-tiles
ST = S // P        # 16 s-tiles
MT = MG // P       # 4 m-tiles == head pairs
SC = S // 512      # 4 s-chunks
W_SCALE = 1024.0

_NC_CACHE = {}


def _build_nc():
    import concourse.mybir as mybir
    from concourse import bacc
    from concourse.tile import TileContext

    f32 = mybir.dt.float32
    f16 = mybir.dt.float16
    f8e4 = mybir.dt.float8e4
    DR = mybir.MatmulPerfMode.DoubleRow

    nc = bacc.Bacc()
    wT_d = nc.declare_dram_parameter("wT", [KT * P, MG], f16, isOutput=False)
    xT_d = nc.declare_dram_parameter("xT", [SC * KT * P, 512], f16, isOutput=False)
    xn_d = nc.declare_dram_parameter("xn", [S, MG], f8e4, isOutput=False)
    outB_d = nc.declare_dram_parameter("outB", [MT * SC * P, 512], f16, isOutput=True)

    wT_t = wT_d.rearrange("(kt p) m -> p kt m", p=P)             # [128, 8, 512]
    xT_t = xT_d.rearrange("(sc kt p) n -> p sc kt n", sc=SC, kt=KT, p=P)
    xn_t = xn_d.rearrange("(st p) m -> p st m", p=P)             # [128, 16, 512]
    outB_t = outB_d.rearrange("(mt sc p) n -> p mt sc n", mt=MT, sc=SC, p=P)

    with TileContext(nc) as tc:
        with (
            tc.tile_pool(name="big", bufs=1) as big,
            tc.tile_pool(name="gp", bufs=1) as gpool,
            tc.tile_pool(name="stage", bufs=4) as stage,
            tc.tile_pool(name="ps_q", bufs=1, space="PSUM") as ps_q,
            tc.tile_pool(name="ps_g", bufs=2, space="PSUM") as ps_g,
            tc.tile_pool(name="ps_o", bufs=2, space="PSUM") as ps_o,
        ):
            wT_sb = big.tile([P, KT, MG], f16, tag="wT")
            xT_sb = big.tile([P, SC, KT, 512], f16, tag="xT")
            xn_sb = big.tile([P, ST, MG], f8e4, tag="xn")
            q_sb = big.tile([P, MT, S], f16, tag="q")

            # ---- Warmup: spin the PE p-state up while the first DMA lands.
            wu_sb = gpool.tile([P, P], f16, tag="wu", name="wu_sb")
            nc.vector.memset(wu_sb, 0.0)
            for i in range(10):
                wu_ps = ps_o.tile([P, P], f32, tag="pso", name=f"wu_ps{i}")
                nc.tensor.matmul(wu_ps, lhsT=wu_sb, rhs=wu_sb, start=True, stop=True)

            # Gram block-diagonal lhsT tiles; memset early (DVE is idle).
            gbd = []
            for p_i in range(MT):
                g = gpool.tile([P, P], f16, tag=f"g{p_i}", name=f"g{p_i}")
                nc.vector.memset(g, 0.0)
                gbd.append(g)

            # ---- Input DMA ring (Sync engine), exact consumption order.
            for kt in range(KT):
                nc.sync.dma_start(out=wT_sb[:, kt], in_=wT_t[:, kt])
                nc.sync.dma_start(out=xT_sb[:, 0, kt], in_=xT_t[:, 0, kt])
            # sc1 interleaved with xn so the Gram operand lands mid-stream
            # without starving the sc1 projection.
            for i in range(4):
                nc.sync.dma_start(
                    out=xT_sb[:, 1, 2 * i:2 * i + 2], in_=xT_t[:, 1, 2 * i:2 * i + 2]
                )
                nc.sync.dma_start(
                    out=xn_sb[:, 4 * i:4 * i + 4], in_=xn_t[:, 4 * i:4 * i + 4]
                )
            nc.sync.dma_start(out=xT_sb[:, 2], in_=xT_t[:, 2])
            nc.sync.dma_start(out=xT_sb[:, 3], in_=xT_t[:, 3])

            def drain_engine(i):
                return nc.vector if i % 2 == 0 else nc.scalar

            def proj(sc):
                psqs = [
                    ps_q.tile([P, 512], f32, tag=f"psq{mt}", name=f"psq{sc}_{mt}")
                    for mt in range(MT)
                ]
                for kt in range(KT):
                    for mt in range(MT):
                        nc.tensor.matmul(
                            psqs[mt],
                            lhsT=wT_sb[:, kt, mt * P:(mt + 1) * P],
                            rhs=xT_sb[:, sc, kt],
                            start=(kt == 0),
                            stop=(kt == KT - 1),
                        )
                for mt in range(MT):
                    eng = drain_engine(mt)
                    dst = q_sb[:, mt, sc * 512:(sc + 1) * 512]
                    if eng is nc.vector:
                        eng.tensor_copy(out=dst, in_=psqs[mt])
                    else:
                        eng.copy(out=dst, in_=psqs[mt])

            def gram():
                for p_i in range(MT):
                    psg = ps_g.tile([P, P], f32, tag="psg", name=f"psg{p_i}")
                    xp = xn_sb[:, :, p_i * P:(p_i + 1) * P]
                    for i in range(ST // 2):
                        nc.tensor.matmul(
                            psg,
                            lhsT=xp[:, 2 * i:2 * i + 2],
                            rhs=xp[:, 2 * i:2 * i + 2],
                            start=(i == 0),
                            stop=(i == ST // 2 - 1),
                            perf_mode=DR,
                        )
                    nc.vector.tensor_scalar_mul(
                        out=gbd[p_i][0:HD, 0:HD],
                        in0=psg[0:HD, 0:HD],
                        scalar1=1.0 / W_SCALE,
                    )
                    nc.vector.tensor_scalar_mul(
                        out=gbd[p_i][HD:P, HD:P],
                        in0=psg[HD:P, HD:P],
                        scalar1=1.0 / W_SCALE,
                    )

            def out_stage(sc):
                for mt in range(MT):
                    pso = ps_o.tile([P, 512], f32, tag="pso", name=f"pso{sc}_{mt}")
                    nc.tensor.matmul(
                        pso,
                        lhsT=gbd[mt],
                        rhs=q_sb[:, mt, sc * 512:(sc + 1) * 512],
                        start=True,
                        stop=True,
                    )
                    ot = stage.tile([P, 512], f16, tag="ot", name=f"ot{sc}_{mt}")
                    eng = drain_engine(mt + 1)
                    if eng is nc.vector:
                        eng.tensor_copy(out=ot, in_=pso)
                    else:
                        eng.copy(out=ot, in_=pso)
                    nc.gpsimd.dma_start(out=outB_t[:, mt, sc], in_=ot)

            proj(0)
            proj(1)
            gram()
            out_stage(0)
            out_stage(1)
            proj(2)
            out_stage(2)
            proj(3)
            out_stage(3)
    nc.compile()
    return nc


def _get_nc():
    if "nc" not in _NC_CACHE:
        _NC_CACHE["nc"] = _build_nc()
    return _NC_CACHE["nc"]


def make_in_maps(hidden_states, queries_weight):
    import ml_dtypes

    f8e4 = ml_dtypes.float8_e4m3
    hs = np.ascontiguousarray(np.asarray(hidden_states, dtype=np.float32))
    w = np.ascontiguousarray(np.asarray(queries_weight, dtype=np.float32))
    in_maps = []
    for c in range(N_CORES):
        b, hg = divmod(c, 2)
        xb = hs[b]
        xT = np.ascontiguousarray(xb.T)  # [1024, 2048]
        in_maps.append({
            # blocked (sc, kt): rows kt*128..+128, cols sc*512..+512 contiguous
            "xT": np.ascontiguousarray(
                xT.reshape(KT, P, SC, 512).transpose(2, 0, 1, 3).reshape(
                    SC * KT * P, 512
                )
            ).astype(np.float16),
            "xn": np.ascontiguousarray(
                xb[:, hg * MG:(hg + 1) * MG]
            ).astype(f8e4),
            "wT": np.ascontiguousarray(
                w[hg * MG:(hg + 1) * MG, :].T * W_SCALE
            ).astype(np.float16),
        })
    return in_maps


def assemble_output(results):
    out = np.empty((B, S, H), dtype=np.float32)
    for c in range(N_CORES):
        b, hg = divmod(c, 2)
        r = np.asarray(results[c]["outB"])  # [MT*SC*P, 512] f16
        out[b, :, hg * MG:(hg + 1) * MG] = (
            r.reshape(MT, SC, P, 512).transpose(1, 3, 0, 2).reshape(S, MG)
        ).astype(np.float32)
    return out


def kernel(hidden_states, queries_weight):
    from concourse.bass_utils import run_bass_kernel_spmd

    in_maps = make_in_maps(hidden_states, queries_weight)
    res = run_bass_kernel_spmd(
        _get_nc(), in_maps, core_ids=list(range(N_CORES))
    ).results
    return assemble_output(res)


if __name__ == "__main__":
    x = np.random.randn(B, S, H).astype(np.float32)
    w = np.random.randn(H, H).astype(np.float32) * 1e-4
    out = kernel(x, w)
    print(out.shape, out.dtype)
